# revision 9
# baseline (speedup 1.0000x reference)
"""NNLS (nonnegative least squares with free bias) for Trainium2.

Problem: X [2000000, 32] f32, y [2000000, 4] f32.
reference = FISTA on normal equations of A = [X, 1]:
    G = A^T A (33x33), c = A^T y (33x4), then 400 projected-FISTA iters.
Heavy part is the single pass over X to form G -> memory bound.

Strategy (v3, fp8 + raw bass):
  - G ~ 2e6*I is extremely well conditioned, so the device-side X^T X
    tolerates very low input precision: quantizing X to fp8 e4m3
    perturbs the final W by ~7e-4 relative (measured), far inside the
    2e-2 gate. c = X^T y is precision-critical (it scales W 1:1) and
    stays exact on host, as does the tiny 33x33 FISTA solve.
  - Host quantizes X to fp8 and permutes it into the exact SBUF layout
    the matmuls want; each of the 8 cores then DMAs its 8.03 MB shard
    (vs 32.1 MB in f32) -> 4x less HBM traffic.
  - Device compute: one fp8 DoubleRow matmul per "unit" of 1024 rows.
    Layout per unit per partition: 256 B = [i(2), d(4), f(32)] -> the
    matmul AP [128, 2, 128] contracts 256 rows per instruction with
    lhsT = rhs = the unit. Diagonal 32x32 blocks of the [128, 128]
    PSUM accumulator are the true partial sums of X^T X (4 dslices per
    unit); off-diagonal blocks are garbage in fixed positions. All 245
    units accumulate into a single PSUM tile.
  - Raw bass (no TileContext): manual semaphores; 10 chained input DMAs
    keep the DMA engines back-to-back at the 360 GB/s cap; the result
    writeback uses the SWDGE prepare/trigger protocol (kv_writeback
    prepare_only early + trigger after the PSUM->SBUF copy) so the
    descriptor-generation latency is off the critical path.
  - Host: sum diagonal blocks over cores/positions, add the ones
    row/column (column sums via np.sum), run the 33x33 FISTA in f64.
  - Safety net: if the device partials fail cheap host invariants, the
    run is retried on a conservative TileContext build with a plain
    HWDGE output DMA.
"""

import numpy as np
import ml_dtypes

P = 128
D = 32
M = 4
NCORES = 8

# Per-core geometry: unit = 1024 rows laid out [128 part, 2 i, 4 d, 32 f]
# (256 B per partition). 245 units/core = 250880 rows; 8 cores cover the
# 2M rows with 7040 zero-pad rows (zeros contribute nothing to G).
UNITS = 245
ROWS_PER_UNIT = P * 8
ROWS_PER_CORE = UNITS * ROWS_PER_UNIT
TOTAL_ROWS = NCORES * ROWS_PER_CORE
N_ROWS = 2_000_000
LINE = UNITS * 256  # fp8 bytes per partition

# DMA split (in units): big chained transfers keep the DMA engines
# saturated; the small final tiles shrink the post-DMA tail.
DMA_TILES = (32, 32, 32, 32, 32, 32, 32, 16, 4, 1)
assert sum(DMA_TILES) == UNITS

POWER_ITERS = 50
QP_ITERS = 400

_CACHE = {}


def build_nc_raw(dma_tiles=DMA_TILES):
    """Raw-bass per-core module: manual sems, SWDGE-triggered writeback."""
    import concourse.mybir as mybir
    from concourse import bacc

    f32 = mybir.dt.float32
    f8 = mybir.dt.float8e4
    i32 = mybir.dt.int32

    nc = bacc.Bacc(trn_type="TRN2")
    x_in = nc.dram_tensor("x_in", [P, LINE], f8, kind="ExternalInput")
    out_g = nc.dram_tensor("out_g", [1, P, 1, P], f32, kind="ExternalOutput")
    scratch = nc.dram_tensor("scratch", [1, P, 1, P], f32, kind="Internal")

    with (
        nc.semaphore("dma_sem") as dma_sem,
        nc.semaphore("warm_sem") as warm_sem,
        nc.semaphore("pe_sem") as pe_sem,
        nc.semaphore("cp_sem") as cp_sem,
        nc.semaphore("out_sem") as out_sem,
        nc.semaphore("prep_sem") as prep_sem,
        nc.sbuf_tensor("xt", [P, UNITS, 2, P], f8) as xt,
        nc.psum_tensor("ps", [P, P], f32) as ps,
        nc.sbuf_tensor("og", [P, P], f32) as og,
        nc.sbuf_tensor("idx", [P, 1], i32) as idx,
        nc.sbuf_tensor("zt", [P, P], f32) as zt,
        nc.semaphore("z_sem") as z_sem,
    ):
        with nc.Block() as block:

            @block.sync
            def _(sync):
                u0 = 0
                for ti, tu in enumerate(dma_tiles):
                    src = x_in[:, u0 * 256 : (u0 + tu) * 256].rearrange(
                        "p (u i m) -> p u i m", i=2, m=P
                    )
                    sync.dma_start(
                        out=xt[:, u0 : u0 + tu], in_=src
                    ).then_inc(dma_sem, 16)
                    u0 += tu
                    if ti == 0:
                        # Zero-fill out_g early via HWDGE: warms the output
                        # mapping for the SWDGE writeback (whose first-exec
                        # write otherwise vanishes) and turns any residual
                        # writeback failure into a clean all-zeros signature
                        # for the host-side partial checks.
                        sync.wait_ge(z_sem, 1)
                        sync.dma_start(
                            out=out_g[:, :, :, :],
                            in_=zt[:].rearrange("p (a b n) -> p a b n", a=1, b=1),
                        ).then_inc(z_sem, 16)

            @block.tensor
            def _(tensor):
                # Semaphores are NOT cleared by any preamble here
                # (target_bir_lowering=False), and stale-high values from a
                # previously loaded program would let waits race through on
                # garbage data. Each sem is therefore cleared by the engine
                # that WAITS on it, at the head of its own in-order stream:
                # the matching producer increments land microseconds later,
                # so the clear always precedes both wait and increment.
                tensor.sem_clear(dma_sem)
                u0 = 0
                for ti, tu in enumerate(dma_tiles):
                    tensor.wait_ge(dma_sem, 16 * (ti + 1))
                    for u in range(u0, u0 + tu):
                        mm = tensor.matmul(
                            ps[:],
                            xt[:, u],
                            xt[:, u],
                            start=(u == 0),
                            stop=(u == UNITS - 1),
                            perf_mode=mybir.MatmulPerfMode.DoubleRow,
                        )
                    u0 += tu
                mm.then_inc(pe_sem, 1)

            @block.vector
            def _(vector):
                vector.sem_clear(pe_sem)
                vector.memset(zt[:], 0.0).then_inc(z_sem, 1)
                vector.wait_ge(pe_sem, 1)
                vector.tensor_copy(og[:], ps[:]).then_inc(cp_sem, 1)

            @block.gpsimd
            def _(gpsimd):
                # Reset SWDGE/DGE state (ring doorbell pointers can be stale
                # from a previously loaded program on the first execution).
                gpsimd.dma_reset(nc._kernel_sem_range)
                gpsimd.sem_clear(cp_sem)
                gpsimd.sem_clear(out_sem)
                gpsimd.sem_clear(prep_sem)
                gpsimd.sem_clear(warm_sem)
                gpsimd.memset(idx[:], 0)
                # Warm-up SWDGE writeback to a scratch buffer: the first
                # SWDGE op of a fresh NEFF execution can lose its write on
                # some cores; this dummy absorbs that so the real writeback
                # below is the second, reliable op. Runs hidden under the
                # input stream (og content is garbage here - irrelevant).
                gpsimd.kv_writeback(
                    scratch[:, :, :, :],
                    og[:].rearrange("p (a b n) -> p a b n", a=1, b=1),
                    idx[:],
                    prepare_only=True,
                    sem=warm_sem,
                ).then_inc(prep_sem, 1)
                gpsimd.wait_ge(prep_sem, 1)
                gpsimd.trigger_dma(count=1)
                gpsimd.wait_ge(warm_sem, 16)
                # Writeback og -> out_g as [batch=1, dhi=128, dho=1,
                # n_ctx=128] at ctx 0: out_g[0, p, 0, :] = og[p, :].
                # prepare_only generates descriptors early (hidden under
                # the input stream); trigger fires them after the copy.
                gpsimd.kv_writeback(
                    out_g[:, :, :, :],
                    og[:].rearrange("p (a b n) -> p a b n", a=1, b=1),
                    idx[:],
                    prepare_only=True,
                    sem=out_sem,
                ).then_inc(prep_sem, 1)
                gpsimd.wait_ge(prep_sem, 2)
                gpsimd.wait_ge(cp_sem, 1)
                gpsimd.trigger_dma(count=1)
                gpsimd.wait_ge(out_sem, 16)

    nc.compile()
    return nc


def build_nc_tile(dma_tiles=DMA_TILES):
    """Conservative fallback: TileContext build, plain HWDGE output DMA."""
    import concourse.mybir as mybir
    from concourse import bacc
    from concourse.tile import TileContext

    f32 = mybir.dt.float32
    f8 = mybir.dt.float8e4

    nc = bacc.Bacc(trn_type="TRN2")
    x_in = nc.dram_tensor("x_in", [P, LINE], f8, kind="ExternalInput")
    out_g = nc.dram_tensor("out_g", [1, P, 1, P], f32, kind="ExternalOutput")

    with TileContext(nc) as tc:
        with (
            tc.tile_pool(name="xp", bufs=1) as xpool,
            tc.tile_pool(name="ps", bufs=1, space="PSUM") as pspool,
        ):
            xt = xpool.tile([P, UNITS, 2, P], f8)
            ps = pspool.tile([P, P], f32)
            u0 = 0
            for tu in dma_tiles:
                src = x_in[:, u0 * 256 : (u0 + tu) * 256].rearrange(
                    "p (u i m) -> p u i m", i=2, m=P
                )
                nc.sync.dma_start(out=xt[:, u0 : u0 + tu], in_=src)
                for u in range(u0, u0 + tu):
                    nc.tensor.matmul(
                        ps[:],
                        xt[:, u],
                        xt[:, u],
                        start=(u == 0),
                        stop=(u == UNITS - 1),
                        perf_mode=mybir.MatmulPerfMode.DoubleRow,
                    )
                u0 += tu
            og = xpool.tile([P, P], f32)
            nc.vector.tensor_copy(og[:], ps[:])
            nc.sync.dma_start(
                out=out_g[:, :, :, :],
                in_=og[:].rearrange("p (a b n) -> p a b n", a=1, b=1),
            )
    nc.compile()
    return nc


def get_nc(variant="raw"):
    key = (variant, DMA_TILES)
    if key not in _CACHE:
        _CACHE[key] = (
            build_nc_raw(DMA_TILES) if variant == "raw" else build_nc_tile(DMA_TILES)
        )
    return _CACHE[key]


def make_shards(X):
    """Quantize X to fp8 e4m3 and permute into the device layout.

    Row -> (core, unit, partition, i, d) mapping is an arbitrary
    bijection (G sums over all rows), chosen so the host transform is
    a single cheap block transpose.
    """
    xq = np.zeros((TOTAL_ROWS, D), dtype=ml_dtypes.float8_e4m3)
    xq[: X.shape[0]] = X.astype(ml_dtypes.float8_e4m3)
    xv = xq.reshape(NCORES, UNITS, P, 8, D).transpose(0, 2, 1, 3, 4)
    return [
        np.ascontiguousarray(xv[c]).reshape(P, LINE) for c in range(NCORES)
    ]


def reduce_partials(results):
    """Sum the diagonal 32x32 blocks of the per-core PSUM dumps."""
    g = np.zeros((D, D), dtype=np.float64)
    for res in results:
        gg = res["out_g"].reshape(P, P).astype(np.float64)
        for a in range(4):
            g += gg[32 * a : 32 * a + 32, 32 * a : 32 * a + 32]
    return g


def host_xty(X, y):
    """Exact-ish X^T y on host: chunked f32 sgemm, f64 accumulation.

    c scales the solution 1:1, so its precision dominates the final
    error; keeping it on host removes it from the quantization budget
    and saves 11% of device HBM traffic."""
    c = np.zeros((D, M), dtype=np.float64)
    ch = 250000
    for i in range(0, X.shape[0], ch):
        c += (X[i : i + ch].T @ y[i : i + ch]).astype(np.float64)
    return c


def solve_qp(G, c):
    """Replicates the reference FISTA solve (f64). G [33,33], c [33,4]."""
    d = D
    v = np.ones(d + 1) / np.sqrt(d + 1)
    for _ in range(POWER_ITERS):
        w = G @ v
        v = w / np.linalg.norm(w)
    L = v @ (G @ v)
    step = 1.0 / L

    Z = np.zeros((d + 1, M))
    Y = Z.copy()
    t = 1.0
    for _ in range(QP_ITERS):
        Zn = Y - step * (G @ Y - c)
        Zn[:d] = np.maximum(Zn[:d], 0.0)
        tn = 0.5 * (1.0 + np.sqrt(1.0 + 4.0 * t * t))
        Y = Zn + ((t - 1.0) / tn) * (Zn - Z)
        Z, t = Zn, tn
    return Z


def run_device(X, y, trace=False, variant="raw"):
    """Run the bass kernel on 8 cores; returns (results, BassKernelResults)."""
    from concourse.bass_utils import run_bass_kernel_spmd

    nc = get_nc(variant)
    shards = make_shards(np.ascontiguousarray(X, dtype=np.float32))
    in_maps = [{"x_in": shards[i]} for i in range(NCORES)]
    r = run_bass_kernel_spmd(
        nc, in_maps, core_ids=list(range(NCORES)), trace=trace
    )
    return r.results, r


def _check_partials(g32, X):
    """Cheap host invariants to catch corrupted device G partials.

    c is host-computed (exact), and W is insensitive to small G noise
    (G ~ 2e6*I regularizes it), so these checks only need to catch
    gross corruption. Good fp8 runs: trace rel ~3e-4 (quantization
    bias), asym bitwise ~0."""
    tx = float(np.dot(X.ravel(), X.ravel()))
    tr_rel = abs(g32.trace() - tx) / max(tx, 1.0)
    asym = np.abs(g32 - g32.T).max()
    ok = tr_rel < 1.5e-3 and asym < 10.0
    return ok, (tr_rel, asym)


def kernel(X, y):
    X = np.asarray(X)
    y = np.asarray(y)

    g32 = None
    for attempt, variant in enumerate(("raw", "raw", "tile")):
        try:
            results, _ = run_device(X, y, variant=variant)
        except Exception as e:
            if attempt == 2:
                raise
            print(f"kernel: device run failed ({variant}, attempt {attempt}): "
                  f"{e}; retrying")
            continue
        g32 = reduce_partials(results)
        ok, stats = _check_partials(g32, X)
        if ok:
            break
        print(f"kernel: partial-sum check failed ({variant}, attempt {attempt}): "
              f"trace_rel={stats[0]:.2e} asym={stats[1]:.2f}")

    sx = X.sum(axis=0, dtype=np.float64)
    sy = y.sum(axis=0, dtype=np.float64)
    n = np.float64(X.shape[0])

    G = np.zeros((D + 1, D + 1))
    G[:D, :D] = g32
    G[:D, D] = sx
    G[D, :D] = sx
    G[D, D] = n
    c = np.zeros((D + 1, M))
    c[:D] = host_xty(X, y)
    c[D] = sy

    Z = solve_qp(G, c)
    return Z[:D].astype(np.float32)


# revision 10
# speedup vs baseline: 1.0067x; 1.0067x over previous
"""NNLS (nonnegative least squares with free bias) for Trainium2.

Problem: X [2000000, 32] f32, y [2000000, 4] f32.
reference = FISTA on normal equations of A = [X, 1]:
    G = A^T A (33x33), c = A^T y (33x4), then 400 projected-FISTA iters.
Heavy part is the single pass over X to form G -> memory bound.

Strategy (v3, fp8 + raw bass):
  - G ~ 2e6*I is extremely well conditioned, so the device-side X^T X
    tolerates very low input precision: quantizing X to fp8 e4m3
    perturbs the final W by ~7e-4 relative (measured), far inside the
    2e-2 gate. c = X^T y is precision-critical (it scales W 1:1) and
    stays exact on host, as does the tiny 33x33 FISTA solve.
  - Host quantizes X to fp8 and permutes it into the exact SBUF layout
    the matmuls want; each of the 8 cores then DMAs its 8.03 MB shard
    (vs 32.1 MB in f32) -> 4x less HBM traffic.
  - Device compute: one fp8 DoubleRow matmul per "unit" of 1024 rows.
    Layout per unit per partition: 256 B = [i(2), d(4), f(32)] -> the
    matmul AP [128, 2, 128] contracts 256 rows per instruction with
    lhsT = rhs = the unit. Diagonal 32x32 blocks of the [128, 128]
    PSUM accumulator are the true partial sums of X^T X (4 dslices per
    unit); off-diagonal blocks are garbage in fixed positions. All 245
    units accumulate into a single PSUM tile.
  - Raw bass (no TileContext): manual semaphores; 10 chained input DMAs
    keep the DMA engines back-to-back at the 360 GB/s cap; the result
    writeback uses the SWDGE prepare/trigger protocol (kv_writeback
    prepare_only early + trigger after the PSUM->SBUF copy) so the
    descriptor-generation latency is off the critical path.
  - Host: sum diagonal blocks over cores/positions, add the ones
    row/column (column sums via np.sum), run the 33x33 FISTA in f64.
  - Safety net: if the device partials fail cheap host invariants, the
    run is retried on a conservative TileContext build with a plain
    HWDGE output DMA.
"""

import numpy as np
import ml_dtypes

P = 128
D = 32
M = 4
NCORES = 8

# Per-core geometry: unit = 1024 rows laid out [128 part, 2 i, 4 d, 32 f]
# (256 B per partition). 245 units/core = 250880 rows; 8 cores cover the
# 2M rows with 7040 zero-pad rows (zeros contribute nothing to G).
UNITS = 245
ROWS_PER_UNIT = P * 8
ROWS_PER_CORE = UNITS * ROWS_PER_UNIT
TOTAL_ROWS = NCORES * ROWS_PER_CORE
N_ROWS = 2_000_000
LINE = UNITS * 256  # fp8 bytes per partition

# DMA split (in units): big chained transfers keep the DMA engines
# saturated; the small final tiles shrink the post-DMA tail.
DMA_TILES = (32, 32, 32, 32, 32, 32, 32, 16, 4, 1)
assert sum(DMA_TILES) == UNITS

POWER_ITERS = 50
QP_ITERS = 400

_CACHE = {}


def build_nc_raw(dma_tiles=DMA_TILES):
    """Raw-bass per-core module: manual sems, SWDGE-triggered writeback."""
    import concourse.mybir as mybir
    from concourse import bacc

    f32 = mybir.dt.float32
    f8 = mybir.dt.float8e4
    i32 = mybir.dt.int32

    nc = bacc.Bacc(trn_type="TRN2")
    x_in = nc.dram_tensor("x_in", [P, LINE], f8, kind="ExternalInput")
    out_g = nc.dram_tensor("out_g", [1, P, 1, P], f32, kind="ExternalOutput")
    scratch = nc.dram_tensor("scratch", [1, P, 1, P], f32, kind="Internal")

    with (
        nc.semaphore("dma_sem") as dma_sem,
        nc.semaphore("warm_sem") as warm_sem,
        nc.semaphore("pe_sem") as pe_sem,
        nc.semaphore("cp_sem") as cp_sem,
        nc.semaphore("out_sem") as out_sem,
        nc.semaphore("prep_sem") as prep_sem,
        nc.sbuf_tensor("xt", [P, UNITS, 2, P], f8) as xt,
        nc.psum_tensor("ps", [P, P], f32) as ps,
        nc.sbuf_tensor("og", [P, P], f32) as og,
        nc.sbuf_tensor("idx", [P, 1], i32) as idx,
    ):
        with nc.Block() as block:

            @block.sync
            def _(sync):
                u0 = 0
                for tu in dma_tiles:
                    src = x_in[:, u0 * 256 : (u0 + tu) * 256].rearrange(
                        "p (u i m) -> p u i m", i=2, m=P
                    )
                    sync.dma_start(
                        out=xt[:, u0 : u0 + tu], in_=src
                    ).then_inc(dma_sem, 16)
                    u0 += tu

            @block.tensor
            def _(tensor):
                # Semaphores are NOT cleared by any preamble here
                # (target_bir_lowering=False), and stale-high values from a
                # previously loaded program would let waits race through on
                # garbage data. Each sem is therefore cleared by the engine
                # that WAITS on it, at the head of its own in-order stream:
                # the matching producer increments land microseconds later,
                # so the clear always precedes both wait and increment.
                tensor.sem_clear(dma_sem)
                u0 = 0
                for ti, tu in enumerate(dma_tiles):
                    tensor.wait_ge(dma_sem, 16 * (ti + 1))
                    for u in range(u0, u0 + tu):
                        mm = tensor.matmul(
                            ps[:],
                            xt[:, u],
                            xt[:, u],
                            start=(u == 0),
                            stop=(u == UNITS - 1),
                            perf_mode=mybir.MatmulPerfMode.DoubleRow,
                        )
                    u0 += tu
                mm.then_inc(pe_sem, 1)

            @block.vector
            def _(vector):
                vector.sem_clear(pe_sem)
                vector.wait_ge(pe_sem, 1)
                vector.tensor_copy(og[:], ps[:]).then_inc(cp_sem, 1)

            @block.gpsimd
            def _(gpsimd):
                # Reset SWDGE/DGE state (ring doorbell pointers can be stale
                # from a previously loaded program on the first execution).
                gpsimd.dma_reset(nc._kernel_sem_range)
                gpsimd.sem_clear(cp_sem)
                gpsimd.sem_clear(out_sem)
                gpsimd.sem_clear(prep_sem)
                gpsimd.sem_clear(warm_sem)
                gpsimd.memset(idx[:], 0)
                # Warm-up SWDGE writeback to a scratch buffer: the first
                # SWDGE op of a fresh NEFF execution can lose its write on
                # some cores; this dummy absorbs that so the real writeback
                # below is the second, reliable op. Runs hidden under the
                # input stream (og content is garbage here - irrelevant).
                gpsimd.kv_writeback(
                    scratch[:, :, :, :],
                    og[:].rearrange("p (a b n) -> p a b n", a=1, b=1),
                    idx[:],
                    prepare_only=True,
                    sem=warm_sem,
                ).then_inc(prep_sem, 1)
                gpsimd.wait_ge(prep_sem, 1)
                gpsimd.trigger_dma(count=1)
                gpsimd.wait_ge(warm_sem, 16)
                # Writeback og -> out_g as [batch=1, dhi=128, dho=1,
                # n_ctx=128] at ctx 0: out_g[0, p, 0, :] = og[p, :].
                # prepare_only generates descriptors early (hidden under
                # the input stream); trigger fires them after the copy.
                gpsimd.kv_writeback(
                    out_g[:, :, :, :],
                    og[:].rearrange("p (a b n) -> p a b n", a=1, b=1),
                    idx[:],
                    prepare_only=True,
                    sem=out_sem,
                ).then_inc(prep_sem, 1)
                gpsimd.wait_ge(prep_sem, 2)
                gpsimd.wait_ge(cp_sem, 1)
                gpsimd.trigger_dma(count=1)
                gpsimd.wait_ge(out_sem, 16)

    nc.compile()
    return nc


def build_nc_tile(dma_tiles=DMA_TILES):
    """Conservative fallback: TileContext build, plain HWDGE output DMA."""
    import concourse.mybir as mybir
    from concourse import bacc
    from concourse.tile import TileContext

    f32 = mybir.dt.float32
    f8 = mybir.dt.float8e4

    nc = bacc.Bacc(trn_type="TRN2")
    x_in = nc.dram_tensor("x_in", [P, LINE], f8, kind="ExternalInput")
    out_g = nc.dram_tensor("out_g", [1, P, 1, P], f32, kind="ExternalOutput")

    with TileContext(nc) as tc:
        with (
            tc.tile_pool(name="xp", bufs=1) as xpool,
            tc.tile_pool(name="ps", bufs=1, space="PSUM") as pspool,
        ):
            xt = xpool.tile([P, UNITS, 2, P], f8)
            ps = pspool.tile([P, P], f32)
            u0 = 0
            for tu in dma_tiles:
                src = x_in[:, u0 * 256 : (u0 + tu) * 256].rearrange(
                    "p (u i m) -> p u i m", i=2, m=P
                )
                nc.sync.dma_start(out=xt[:, u0 : u0 + tu], in_=src)
                for u in range(u0, u0 + tu):
                    nc.tensor.matmul(
                        ps[:],
                        xt[:, u],
                        xt[:, u],
                        start=(u == 0),
                        stop=(u == UNITS - 1),
                        perf_mode=mybir.MatmulPerfMode.DoubleRow,
                    )
                u0 += tu
            og = xpool.tile([P, P], f32)
            nc.vector.tensor_copy(og[:], ps[:])
            nc.sync.dma_start(
                out=out_g[:, :, :, :],
                in_=og[:].rearrange("p (a b n) -> p a b n", a=1, b=1),
            )
    nc.compile()
    return nc


def get_nc(variant="raw"):
    key = (variant, DMA_TILES)
    if key not in _CACHE:
        _CACHE[key] = (
            build_nc_raw(DMA_TILES) if variant == "raw" else build_nc_tile(DMA_TILES)
        )
    return _CACHE[key]


def make_shards(X):
    """Quantize X to fp8 e4m3 and permute into the device layout.

    Row -> (core, unit, partition, i, d) mapping is an arbitrary
    bijection (G sums over all rows), chosen so the host transform is
    a single cheap block transpose.
    """
    xq = np.zeros((TOTAL_ROWS, D), dtype=ml_dtypes.float8_e4m3)
    xq[: X.shape[0]] = X.astype(ml_dtypes.float8_e4m3)
    xv = xq.reshape(NCORES, UNITS, P, 8, D).transpose(0, 2, 1, 3, 4)
    return [
        np.ascontiguousarray(xv[c]).reshape(P, LINE) for c in range(NCORES)
    ]


def reduce_partials(results):
    """Sum the diagonal 32x32 blocks of the per-core PSUM dumps."""
    g = np.zeros((D, D), dtype=np.float64)
    for res in results:
        gg = res["out_g"].reshape(P, P).astype(np.float64)
        for a in range(4):
            g += gg[32 * a : 32 * a + 32, 32 * a : 32 * a + 32]
    return g


def host_xty(X, y):
    """Exact-ish X^T y on host: chunked f32 sgemm, f64 accumulation.

    c scales the solution 1:1, so its precision dominates the final
    error; keeping it on host removes it from the quantization budget
    and saves 11% of device HBM traffic."""
    c = np.zeros((D, M), dtype=np.float64)
    ch = 250000
    for i in range(0, X.shape[0], ch):
        c += (X[i : i + ch].T @ y[i : i + ch]).astype(np.float64)
    return c


def solve_qp(G, c):
    """Replicates the reference FISTA solve (f64). G [33,33], c [33,4]."""
    d = D
    v = np.ones(d + 1) / np.sqrt(d + 1)
    for _ in range(POWER_ITERS):
        w = G @ v
        v = w / np.linalg.norm(w)
    L = v @ (G @ v)
    step = 1.0 / L

    Z = np.zeros((d + 1, M))
    Y = Z.copy()
    t = 1.0
    for _ in range(QP_ITERS):
        Zn = Y - step * (G @ Y - c)
        Zn[:d] = np.maximum(Zn[:d], 0.0)
        tn = 0.5 * (1.0 + np.sqrt(1.0 + 4.0 * t * t))
        Y = Zn + ((t - 1.0) / tn) * (Zn - Z)
        Z, t = Zn, tn
    return Z


def run_device(X, y, trace=False, variant="raw"):
    """Run the bass kernel on 8 cores; returns (results, BassKernelResults)."""
    from concourse.bass_utils import run_bass_kernel_spmd

    nc = get_nc(variant)
    shards = make_shards(np.ascontiguousarray(X, dtype=np.float32))
    in_maps = [{"x_in": shards[i]} for i in range(NCORES)]
    r = run_bass_kernel_spmd(
        nc, in_maps, core_ids=list(range(NCORES)), trace=trace
    )
    return r.results, r


def _check_partials(g32, X):
    """Cheap host invariants to catch corrupted device G partials.

    c is host-computed (exact), and W is insensitive to small G noise
    (G ~ 2e6*I regularizes it), so these checks only need to catch
    gross corruption. Good fp8 runs: trace rel ~3e-4 (quantization
    bias), asym bitwise ~0."""
    tx = float(np.dot(X.ravel(), X.ravel()))
    tr_rel = abs(g32.trace() - tx) / max(tx, 1.0)
    asym = np.abs(g32 - g32.T).max()
    ok = tr_rel < 1.5e-3 and asym < 10.0
    return ok, (tr_rel, asym)


def kernel(X, y):
    X = np.asarray(X)
    y = np.asarray(y)

    g32 = None
    for attempt, variant in enumerate(("raw", "raw", "tile")):
        try:
            results, _ = run_device(X, y, variant=variant)
        except Exception as e:
            if attempt == 2:
                raise
            print(f"kernel: device run failed ({variant}, attempt {attempt}): "
                  f"{e}; retrying")
            continue
        g32 = reduce_partials(results)
        ok, stats = _check_partials(g32, X)
        if ok:
            break
        print(f"kernel: partial-sum check failed ({variant}, attempt {attempt}): "
              f"trace_rel={stats[0]:.2e} asym={stats[1]:.2f}")

    sx = X.sum(axis=0, dtype=np.float64)
    sy = y.sum(axis=0, dtype=np.float64)
    n = np.float64(X.shape[0])

    G = np.zeros((D + 1, D + 1))
    G[:D, :D] = g32
    G[:D, D] = sx
    G[D, :D] = sx
    G[D, D] = n
    c = np.zeros((D + 1, M))
    c[:D] = host_xty(X, y)
    c[D] = sy

    Z = solve_qp(G, c)
    return Z[:D].astype(np.float32)
